# revision 7
# baseline (speedup 1.0000x reference)
"""Trainium2 Bass kernel: 3x3 same-padding Conv2D, NCHW.

Input  (16, 64, 128, 128) f32, weights (128, 64, 3, 3) OIHW, bias (128,).
Output (16, 128, 128, 128) f32.

Strategy: data-parallel over batch — 2 images per NeuronCore on 8 cores.
Per core the conv is computed as accumulated TensorEngine matmuls over
(C_in x tap) contractions:

  - The host pre-builds a padded dual layout per image,
    [128, 130, 130]: partitions 0-63 (copy A) hold the zero-padded image
    shifted down one row (A[r] = padded row r-1), partitions 64-127
    (copy B) hold the padded rows directly (B[r] = padded row r).  One
    fully-contiguous DMA stages it in SBUF (single semaphore wait — the
    LDWEIGHTS half of a self-loading fp32r matmul has very few wait
    slots, so producer count per matmul must stay tiny).
  - For an output row group h..h+3 (free size 4*128 = 512 = one PSUM
    bank) and each kw in 0..2:
      MM1 (K=128): taps (kh=0, kw) on copy A + (kh=1, kw) on copy B in a
      single matmul, since B sits exactly one row below A.
      MM2 (K=64):  tap (kh=2, kw) read from copy A two rows down.
    6 matmuls accumulate into one PSUM bank; epilogue adds bias while
    copying PSUM -> SBUF, then one DMA stores 4 output rows.

Tensors are float32r end-to-end (full-rate fp32 streaming on the PE at
N>=256, vs 4 cycles/row for plain fp32).
"""

import sys

if "/opt/trn_rl_repo" not in sys.path:
    sys.path.insert(0, "/opt/trn_rl_repo")

import numpy as np

N_CORES = 8
IMGS_PER_CORE = 2
H = 128
W = 128
CIN = 64
COUT = 128
WPAD = W + 2  # 130: one zero column each side
HPAD = H + 2  # 130 rows (pad row above and below)
ROWS_PER_GROUP = 4  # 4*128 = 512 free elements = one PSUM bank
WB_COLS = 3 * COUT + 3 * COUT + 1  # w1 (384) | w2 (384, rows 0-63) | bias (1)

_cache = {}


def _build_nc():
    import concourse.mybir as mybir
    from concourse import bacc
    from concourse.tile import TileContext

    f32 = mybir.dt.float32
    f32r = mybir.dt.float32r

    nc = bacc.Bacc(target_bir_lowering=False)
    x_d = nc.dram_tensor(
        "x", [IMGS_PER_CORE, 128, HPAD * WPAD], f32r, kind="ExternalInput"
    )
    # packed weights+bias, one DMA:
    #   cols 0..383   : w1[t*64+ci, kw*128+co] = W[co, ci, t, kw], taps kh=t in {0,1}
    #   cols 384..767 : w2[ci, kw*128+co] = W[co, ci, 2, kw] (rows 0..63)
    #   col  768      : bias[co]
    wb_d = nc.dram_tensor("wb", [128, WB_COLS], f32r, kind="ExternalInput")
    out_d = nc.dram_tensor(
        "out", [IMGS_PER_CORE, COUT, H, W], f32, kind="ExternalOutput"
    )

    with TileContext(nc) as tc:
        with (
            tc.tile_pool(name="wpool", bufs=1) as wpool,
            tc.tile_pool(name="xpool", bufs=2) as xpool,
            tc.tile_pool(name="opool", bufs=4) as opool,
            tc.tile_pool(name="pspool", bufs=8, space="PSUM") as pspool,
        ):
            wb_sb = wpool.tile([128, WB_COLS], f32r)
            nc.sync.dma_start(out=wb_sb[:], in_=wb_d[:])
            w1_sb = wb_sb[:, 0 : 3 * COUT]
            w2_sb = wb_sb[0:CIN, 3 * COUT : 6 * COUT]
            b_sb = wb_sb[:, 6 * COUT : 6 * COUT + 1].bitcast(f32)

            for img in range(IMGS_PER_CORE):
                X = xpool.tile([128, HPAD * WPAD], f32r)
                nc.sync.dma_start(out=X[:], in_=x_d[img])
                X3 = X.rearrange("p (r c) -> p r c", c=WPAD)

                for h in range(0, H, ROWS_PER_GROUP):
                    ps = pspool.tile([COUT, ROWS_PER_GROUP * W], f32)
                    for kw in range(3):
                        # taps (kh=0, kw) + (kh=1, kw), K = 128
                        nc.tensor.matmul(
                            ps[:],
                            w1_sb[:, kw * COUT : (kw + 1) * COUT],
                            X3[:, h : h + ROWS_PER_GROUP, kw : kw + W],
                            start=(kw == 0),
                            stop=False,
                        )
                    for kw in range(3):
                        # tap (kh=2, kw), K = 64 on copy A rows h+2..h+5
                        nc.tensor.matmul(
                            ps[:],
                            w2_sb[:, kw * COUT : (kw + 1) * COUT],
                            X3[0:CIN, h + 2 : h + 2 + ROWS_PER_GROUP, kw : kw + W],
                            start=False,
                            stop=(kw == 2),
                        )
                    ob = opool.tile([COUT, ROWS_PER_GROUP * W], f32)
                    # bias-add while evacuating PSUM; alternate engines so
                    # ScalarE and VectorE each carry half the epilogue.
                    if (h // ROWS_PER_GROUP) % 2 == 0:
                        nc.scalar.add(ob[:], ps[:], b_sb)
                    else:
                        nc.vector.tensor_scalar_add(ob[:], ps[:], b_sb)
                    ob3 = ob.rearrange("p (r c) -> p r c", c=W)
                    nc.sync.dma_start(
                        out=out_d[img, :, h : h + ROWS_PER_GROUP, :], in_=ob3[:]
                    )
    nc.compile()
    return nc


def _get_nc():
    if "nc" not in _cache:
        _cache["nc"] = _build_nc()
    return _cache["nc"]


def _make_dual(images):
    """images: [n, 64, 128, 128] -> [n, 128, HPAD*WPAD] dual padded layout."""
    n = images.shape[0]
    zp = np.zeros((n, CIN, HPAD, WPAD), dtype=np.float32)
    zp[:, :, 1 : H + 1, 1 : W + 1] = images  # padded rows 0..129
    dual = np.empty((n, 128, HPAD, WPAD), dtype=np.float32)
    dual[:, 0:CIN] = zp  # A[r] = padded row r-1 shape-wise (row r of zp)
    dual[:, CIN:128, 0 : HPAD - 1] = zp[:, :, 1:HPAD]  # B[r] = padded row r
    dual[:, CIN:128, HPAD - 1] = 0.0  # B row 129 unread
    return np.ascontiguousarray(dual.reshape(n, 128, HPAD * WPAD))


def _prepare_in_maps(input_tensor, weights, bias):
    input_tensor = np.asarray(input_tensor, dtype=np.float32)
    weights = np.asarray(weights, dtype=np.float32)
    bias = np.asarray(bias, dtype=np.float32)
    wb = np.zeros((128, WB_COLS), dtype=np.float32)
    # [co, ci, kh, kw] -> w1[t*64+ci, kw*128+co], w2[ci, kw*128+co]
    wb[:, 0 : 3 * COUT] = (
        weights[:, :, 0:2, :].transpose(2, 1, 3, 0).reshape(128, 3 * COUT)
    )
    wb[0:CIN, 3 * COUT : 6 * COUT] = (
        weights[:, :, 2, :].transpose(1, 2, 0).reshape(CIN, 3 * COUT)
    )
    wb[:, 6 * COUT] = bias
    in_maps = []
    for c in range(N_CORES):
        shard = _make_dual(input_tensor[c * IMGS_PER_CORE : (c + 1) * IMGS_PER_CORE])
        in_maps.append({"x": shard, "wb": wb})
    return in_maps


def _gather(results):
    return np.concatenate([results[c]["out"] for c in range(N_CORES)], axis=0)


def kernel(input_tensor, weights, bias):
    from concourse.bass_utils import run_bass_kernel_spmd

    nc = _get_nc()
    in_maps = _prepare_in_maps(input_tensor, weights, bias)
    res = run_bass_kernel_spmd(nc, in_maps, core_ids=list(range(N_CORES)))
    return _gather(res.results)


# revision 8
# speedup vs baseline: 1.0936x; 1.0936x over previous
"""Trainium2 Bass kernel: 3x3 same-padding Conv2D, NCHW.

Input  (16, 64, 128, 128) f32, weights (128, 64, 3, 3) OIHW, bias (128,).
Output (16, 128, 128, 128) f32.

Strategy: data-parallel over batch — 2 images per NeuronCore on 8 cores.
Per core the conv is computed as accumulated TensorEngine matmuls over
(C_in x tap) contractions:

  - The host pre-builds a padded dual layout per image,
    [128, 130, 130]: partitions 0-63 (copy A) hold the zero-padded image
    shifted down one row (A[r] = padded row r-1), partitions 64-127
    (copy B) hold the padded rows directly (B[r] = padded row r).  One
    fully-contiguous DMA stages it in SBUF (single semaphore wait — the
    LDWEIGHTS half of a self-loading fp32r matmul has very few wait
    slots, so producer count per matmul must stay tiny).
  - For an output row group h..h+3 (free size 4*128 = 512 = one PSUM
    bank) and each kw in 0..2:
      MM1 (K=128): taps (kh=0, kw) on copy A + (kh=1, kw) on copy B in a
      single matmul, since B sits exactly one row below A.
      MM2 (K=64):  tap (kh=2, kw) read from copy A two rows down.
    6 matmuls accumulate into one PSUM bank; epilogue adds bias while
    copying PSUM -> SBUF, then one DMA stores 4 output rows.

Tensors are float32r end-to-end (full-rate fp32 streaming on the PE at
N>=256, vs 4 cycles/row for plain fp32).
"""

import sys

if "/opt/trn_rl_repo" not in sys.path:
    sys.path.insert(0, "/opt/trn_rl_repo")

import numpy as np

N_CORES = 8
IMGS_PER_CORE = 2
H = 128
W = 128
CIN = 64
COUT = 128
WPAD = W + 2  # 130: one zero column each side
HPAD = H + 2  # 130 rows (pad row above and below)
ROWS_PER_GROUP = 4  # 4*128 = 512 free elements = one PSUM bank
WB_COLS = 3 * COUT + 3 * COUT + 1  # w1 (384) | w2 (384, rows 0-63) | bias (1)

# "f32r": fp32 storage, TF32-like matmul (rel err ~2e-4, ~3 PE cycles/row)
# "bf16": bf16 operands via casting DMA (rel err ~3e-3, 1 PE cycle/row + FWL)
DTYPE_MODE = "bf16"

_cache = {}


def _build_nc(mode=None):
    import concourse.mybir as mybir
    from concourse import bacc
    from concourse.tile import TileContext

    mode = mode or DTYPE_MODE
    f32 = mybir.dt.float32
    f32r = mybir.dt.float32r
    cdt = f32r if mode == "f32r" else mybir.dt.bfloat16

    nc = bacc.Bacc(target_bir_lowering=False)
    x_d = nc.dram_tensor(
        "x", [IMGS_PER_CORE, 128, HPAD * WPAD], f32r, kind="ExternalInput"
    )
    # packed weights+bias, one DMA:
    #   cols 0..383   : w1[t*64+ci, kw*128+co] = W[co, ci, t, kw], taps kh=t in {0,1}
    #   cols 384..767 : w2[ci, kw*128+co] = W[co, ci, 2, kw] (rows 0..63)
    #   col  768      : bias[co]
    wb_d = nc.dram_tensor("wb", [128, WB_COLS], f32r, kind="ExternalInput")
    out_d = nc.dram_tensor(
        "out", [IMGS_PER_CORE, COUT, H, W], f32, kind="ExternalOutput"
    )

    with TileContext(nc) as tc:
        with (
            tc.tile_pool(name="wpool", bufs=1) as wpool,
            tc.tile_pool(name="xpool", bufs=2) as xpool,
            tc.tile_pool(name="opool", bufs=4) as opool,
            tc.tile_pool(name="pspool", bufs=8, space="PSUM") as pspool,
        ):
            wb_sb = wpool.tile([128, WB_COLS], cdt)
            # gpsimd DMA casts f32 -> bf16 in flight; sync DMA for pure copy
            wdma = nc.gpsimd if cdt != f32r else nc.sync
            wdma.dma_start(out=wb_sb[:], in_=wb_d[:])
            w1_sb = wb_sb[:, 0 : 3 * COUT]
            w2_sb = wb_sb[0:CIN, 3 * COUT : 6 * COUT]
            if mode == "f32r":
                b_sb = wb_sb[:, 6 * COUT : 6 * COUT + 1].bitcast(f32)
            else:
                b_f32 = wpool.tile([COUT, 1], f32)
                nc.sync.dma_start(
                    out=b_f32[:], in_=wb_d[:, 6 * COUT : 6 * COUT + 1].bitcast(f32)
                )
                b_sb = b_f32[:]

            for img in range(IMGS_PER_CORE):
                X = xpool.tile([128, HPAD * WPAD], cdt)
                xdma = nc.gpsimd if cdt != f32r else nc.sync
                xdma.dma_start(out=X[:], in_=x_d[img])
                X3 = X.rearrange("p (r c) -> p r c", c=WPAD)

                for h in range(0, H, ROWS_PER_GROUP):
                    ps = pspool.tile([COUT, ROWS_PER_GROUP * W], f32)
                    for kw in range(3):
                        # taps (kh=0, kw) + (kh=1, kw), K = 128
                        nc.tensor.matmul(
                            ps[:],
                            w1_sb[:, kw * COUT : (kw + 1) * COUT],
                            X3[:, h : h + ROWS_PER_GROUP, kw : kw + W],
                            start=(kw == 0),
                            stop=False,
                        )
                    for kw in range(3):
                        # tap (kh=2, kw), K = 64 on copy A rows h+2..h+5
                        nc.tensor.matmul(
                            ps[:],
                            w2_sb[:, kw * COUT : (kw + 1) * COUT],
                            X3[0:CIN, h + 2 : h + 2 + ROWS_PER_GROUP, kw : kw + W],
                            start=False,
                            stop=(kw == 2),
                        )
                    ob = opool.tile([COUT, ROWS_PER_GROUP * W], f32)
                    # bias-add while evacuating PSUM; alternate engines so
                    # ScalarE and VectorE each carry half the epilogue.
                    if (h // ROWS_PER_GROUP) % 2 == 0:
                        nc.scalar.add(ob[:], ps[:], b_sb)
                    else:
                        nc.vector.tensor_scalar_add(ob[:], ps[:], b_sb)
                    ob3 = ob.rearrange("p (r c) -> p r c", c=W)
                    nc.sync.dma_start(
                        out=out_d[img, :, h : h + ROWS_PER_GROUP, :], in_=ob3[:]
                    )
    nc.compile()
    return nc


def _get_nc(mode=None):
    mode = mode or DTYPE_MODE
    if mode not in _cache:
        _cache[mode] = _build_nc(mode)
    return _cache[mode]


def _make_dual(images):
    """images: [n, 64, 128, 128] -> [n, 128, HPAD*WPAD] dual padded layout."""
    n = images.shape[0]
    zp = np.zeros((n, CIN, HPAD, WPAD), dtype=np.float32)
    zp[:, :, 1 : H + 1, 1 : W + 1] = images  # padded rows 0..129
    dual = np.empty((n, 128, HPAD, WPAD), dtype=np.float32)
    dual[:, 0:CIN] = zp  # A[r] = padded row r-1 shape-wise (row r of zp)
    dual[:, CIN:128, 0 : HPAD - 1] = zp[:, :, 1:HPAD]  # B[r] = padded row r
    dual[:, CIN:128, HPAD - 1] = 0.0  # B row 129 unread
    return np.ascontiguousarray(dual.reshape(n, 128, HPAD * WPAD))


def _prepare_in_maps(input_tensor, weights, bias):
    input_tensor = np.asarray(input_tensor, dtype=np.float32)
    weights = np.asarray(weights, dtype=np.float32)
    bias = np.asarray(bias, dtype=np.float32)
    wb = np.zeros((128, WB_COLS), dtype=np.float32)
    # [co, ci, kh, kw] -> w1[t*64+ci, kw*128+co], w2[ci, kw*128+co]
    wb[:, 0 : 3 * COUT] = (
        weights[:, :, 0:2, :].transpose(2, 1, 3, 0).reshape(128, 3 * COUT)
    )
    wb[0:CIN, 3 * COUT : 6 * COUT] = (
        weights[:, :, 2, :].transpose(1, 2, 0).reshape(CIN, 3 * COUT)
    )
    wb[:, 6 * COUT] = bias
    in_maps = []
    for c in range(N_CORES):
        shard = _make_dual(input_tensor[c * IMGS_PER_CORE : (c + 1) * IMGS_PER_CORE])
        in_maps.append({"x": shard, "wb": wb})
    return in_maps


def _gather(results):
    return np.concatenate([results[c]["out"] for c in range(N_CORES)], axis=0)


def kernel(input_tensor, weights, bias):
    from concourse.bass_utils import run_bass_kernel_spmd

    nc = _get_nc()
    in_maps = _prepare_in_maps(input_tensor, weights, bias)
    res = run_bass_kernel_spmd(nc, in_maps, core_ids=list(range(N_CORES)))
    return _gather(res.results)
